# revision 1
# baseline (speedup 1.0000x reference)
"""BiAttention (BiDAF-style) kernel for Trainium2, 8 NeuronCores.

Reference math (T=4096, d=512):
    context  = x[0,0]; question = x[1,0]
    S[i,j]   = w1.c_i + w2.q_j + (c_i*w3).q_j
    A        = softmax_j(S)          # w1.c_i is constant per row -> cancels
    U_A      = A @ question
    b        = max_j A[i,j]          # == max_j E[i,j] / Z_i  with E=exp(S)
    h        = b @ context           # global over T -> one AllReduce
    G        = [context, U_A, context*U_A, context*h]

Sharding: context rows (and rows of S/A/U_A/G) split across 8 cores
(512 rows each); question replicated; h all-reduced (2 KB).

Per-core schedule:
  phase 1 (per 512-wide j-slab): SWDGE cast-load q slab (fp16), PE-transpose
    to qT, then S = W.T @ qT for all four i-blocks where the stationary
    W[dc] = (c*w3).T[dc] + w2[dc] carries the q2 bias for free (because
    sum_dc sum_k w2[k]*qT[dc][k,j] = q2[j]); exp on ACT with fused row-sum
    (Z) accumulation and per-slab row-max partials on DVE.
  phase 2a (per i-block): 1/Z, row-max of E -> b, h-partial matmul into one
    PSUM bank; then the 2 KB h AllReduce launches (hidden under phase 2b).
  phase 2b (per i-block): PE-transpose E -> E.T, U_A = E.T.T @ q_bf scaled
    by 1/Z, write G blocks (including c*h once the AllReduce lands).

All matmul operands are fp16 (1 cycle/row on PE, like bf16, but 4x finer
rounding); accumulation is fp32 in PSUM; stats are fp32.
"""

import numpy as np

import concourse.bass as bass
import concourse.mybir as mybir
import concourse.tile as tile
from concourse import bacc
from concourse.bass_utils import run_bass_kernel_spmd
from concourse.masks import make_identity

F32 = mybir.dt.float32
# fp16 (10-bit mantissa) runs matmuls at the same 1 cycle/row as bf16 but
# with 4x finer rounding; E = exp(S) <= e^6 stays far below fp16 max.
BF16 = mybir.dt.float16
AF = mybir.ActivationFunctionType

T = 4096
D = 512
NCORES = 8
TL = T // NCORES          # 512 local context rows per core
P = 128
NIB = TL // P             # 4 i-blocks of 128 rows
NJT = T // P              # 32 j-tiles of 128
NJS = T // 512            # 8 j-slabs of 512
NDC = D // P              # 4 d-chunks of 128


def build_kernel(collective=True, compile=True):
    nc = bacc.Bacc("TRN2", target_bir_lowering=False, debug=False,
                   num_devices=NCORES if collective else 1)

    c_dram = nc.dram_tensor("c", [TL, D], F32, kind="ExternalInput").ap()
    q_dram = nc.dram_tensor("q", [T, D], F32, kind="ExternalInput").ap()
    w2p_dram = nc.dram_tensor("w2p", [P, NDC], F32, kind="ExternalInput").ap()
    w3p_dram = nc.dram_tensor("w3p", [P, NDC], F32, kind="ExternalInput").ap()
    g_dram = nc.dram_tensor("g", [TL, 4 * D], F32, kind="ExternalOutput").ap()

    with tile.TileContext(nc) as tc:
        _emit(nc, tc, c_dram, q_dram, w2p_dram, w3p_dram, g_dram,
              collective=collective)

    if compile:
        nc.compile()
    return nc


def _emit(nc, tc, c_dram, q_dram, w2p_dram, w3p_dram, g_dram,
          collective=True):
    from contextlib import ExitStack
    ctx = ExitStack()
    consts = ctx.enter_context(tc.tile_pool(name="consts", bufs=1))
    epool = ctx.enter_context(tc.tile_pool(name="epool", bufs=1))
    etpool = ctx.enter_context(tc.tile_pool(name="etpool", bufs=2))
    spool = ctx.enter_context(tc.tile_pool(name="spool", bufs=2, space="PSUM"))
    tppool = ctx.enter_context(tc.tile_pool(name="tppool", bufs=5, space="PSUM"))
    uapool = ctx.enter_context(tc.tile_pool(name="uapool", bufs=1, space="PSUM"))
    stat = ctx.enter_context(tc.tile_pool(name="stat", bufs=4))
    gout = ctx.enter_context(tc.tile_pool(name="gout", bufs=3))
    dram = ctx.enter_context(tc.tile_pool(name="dram", bufs=1, space="DRAM"))

    # ---- prologue ---------------------------------------------------------
    # ident first: it is tiny gpsimd work but gates every PE transpose, and
    # the gpsimd (Pool) queue also generates all SWDGE cast-DMA descriptors.
    q_bf = consts.tile([P, NJS, 4, D], BF16)  # [p, js, k, d] ; jt = 4*js+k
    # c_bf cast-DMA descriptor first: its transfer overlaps ident setup and
    # it gates PE's first work (the cw3T transposes)
    c_bf = consts.tile([P, NIB, D], BF16)  # [p, ib, d]
    nc.gpsimd.dma_start(out=c_bf,
                        in_=c_dram.rearrange("(ib p) d -> p ib d", p=P))
    ident = consts.tile([P, P], BF16)
    make_identity(nc, ident)
    # dummy exp: pull the ~2.7us ACT table load for exp_and_others into the
    # startup DMA-wait window instead of stalling the first real exp
    warm = consts.tile([1, 1], F32)
    nc.vector.memset(warm, 0.0)
    nc.scalar.activation(out=warm, in_=warm, func=AF.Exp)
    # HAM warm-up: dummy matmuls fill the otherwise-idle cold-start DMA wait
    # and bring the PE clock to 2.4 GHz before the real pipeline begins
    wa = consts.tile([P, P], BF16)
    nc.vector.memset(wa, 0.0)
    wb = consts.tile([P, 512], BF16)
    nc.vector.memset(wb, 0.0)
    for wi in range(3):
        wps = tppool.tile([P, 512], F32, tag="tp", name=f"wps{wi}")
        nc.tensor.matmul(wps, lhsT=wa, rhs=wb, start=True, stop=True)

    w2p = consts.tile([P, NDC], F32)
    nc.sync.dma_start(out=w2p, in_=w2p_dram)
    w3p = consts.tile([P, NDC], F32)
    nc.sync.dma_start(out=w3p, in_=w3p_dram)

    qT = []  # qT[dc]: (128 d, 4096 j) bf16
    for dc in range(NDC):
        qT.append(consts.tile([P, T], BF16, tag=f"qT{dc}", name=f"qT{dc}"))

    def emit_slab_transposes(js):
        for dc in range(NDC):
            ps = tppool.tile([P, 512], BF16, tag="tp", name=f"tq{js}{dc}")
            for k in range(4):
                nc.tensor.transpose(ps[:, k * P:(k + 1) * P],
                                    q_bf[:, js, k, dc * P:(dc + 1) * P],
                                    ident)
            nc.vector.tensor_copy(out=qT[dc][:, js * 512:(js + 1) * 512],
                                  in_=ps)

    # ---- context: load f32 ------------------------------------------------
    c_nat = []
    for ib in range(NIB):
        t = consts.tile([P, D], F32, tag=f"c_nat{ib}", name=f"c_nat{ib}")
        nc.sync.dma_start(out=t, in_=c_dram[ib * P:(ib + 1) * P, :])
        c_nat.append(t)

    # cw3T[dc] = (context * w3).T chunk PLUS the w2 bias row-constant:
    # W[dc][k,i] = c[i, dc*128+k]*w3[dc*128+k] + w2[dc*128+k].  Because
    #   sum_dc sum_k w2[dc*128+k] * qT[dc][k,j] = (q @ w2)[j] = q2[j],
    # the S matmul then produces  S = (c*w3) @ q.T + q2  directly — the q2
    # bias costs zero extra matmuls (folded into the stationary operand).
    cw3T = []
    for dc in range(NDC):
        ps = tppool.tile([P, TL], BF16, tag="tp")
        for ib in range(NIB):
            nc.tensor.transpose(ps[:, ib * P:(ib + 1) * P],
                                c_bf[:, ib, dc * P:(dc + 1) * P], ident)
        t = consts.tile([P, TL], BF16, tag=f"cw3T{dc}", name=f"cw3T{dc}")
        nc.scalar.activation(out=t, in_=ps, func=AF.Identity,
                             bias=w2p[:, dc:dc + 1],
                             scale=w3p[:, dc:dc + 1])
        cw3T.append(t)

    # ---- persistent per-i-block E, Z-partial and max-partial buffers -----
    e_sb = []
    zpart = []
    mpart = []
    for ib in range(NIB):
        e_sb.append(epool.tile([P, T], BF16, tag=f"e{ib}", name=f"e{ib}"))
        zpart.append(stat.tile([P, NJS], F32, tag=f"zp{ib}", name=f"zp{ib}"))
        mpart.append(stat.tile([P, NJS], F32, tag=f"mp{ib}", name=f"mp{ib}"))

    # ---- phase 1: per j-slab pipeline ------------------------------------
    for js in range(NJS):
        # cast-load one 512-row slab of question as bf16
        nc.gpsimd.dma_start(
            out=q_bf[:, js],
            in_=q_dram[js * 512:(js + 1) * 512, :]
                .rearrange("(k p) d -> p k d", p=P))
        emit_slab_transposes(js)
        # S (with the q2 bias already folded into cw3T) and E per i-block
        for ib in range(NIB):
            ps = spool.tile([P, 512], F32, tag="s")
            for dc in range(NDC):
                nc.tensor.matmul(ps, lhsT=cw3T[dc][:, ib * P:(ib + 1) * P],
                                 rhs=qT[dc][:, js * 512:(js + 1) * 512],
                                 start=(dc == 0), stop=(dc == NDC - 1))
            nc.scalar.activation(out=e_sb[ib][:, js * 512:(js + 1) * 512],
                                 in_=ps, func=AF.Exp,
                                 accum_out=zpart[ib][:, js:js + 1])
            nc.vector.tensor_reduce(out=mpart[ib][:, js:js + 1],
                                    in_=e_sb[ib][:, js * 512:(js + 1) * 512],
                                    axis=mybir.AxisListType.X,
                                    op=mybir.AluOpType.max)

    # ---- phase 2a: per i-block stats + h partial, launch AllReduce -------
    h_ps = spool.tile([P, NDC], F32, tag="s", name="h_ps")  # takes a freed
    # phase-1 S slot; S psums are all drained by the time phase 2a starts
    zinvs = []
    for ib in range(NIB):
        z = stat.tile([P, 1], F32, tag="z")
        nc.vector.tensor_reduce(out=z, in_=zpart[ib],
                                axis=mybir.AxisListType.X,
                                op=mybir.AluOpType.add)
        zinv = stat.tile([P, 1], F32, tag=f"zinv{ib}", name=f"zinv{ib}")
        nc.vector.reciprocal(out=zinv, in_=z)
        zinvs.append(zinv)
        maxe = stat.tile([P, 1], F32, tag="maxe")
        nc.vector.tensor_reduce(out=maxe, in_=mpart[ib],
                                axis=mybir.AxisListType.X,
                                op=mybir.AluOpType.max)
        b = stat.tile([P, 1], F32, tag="b")
        nc.vector.tensor_mul(out=b, in0=maxe, in1=zinv)
        b_bf = stat.tile([P, 1], BF16, tag="b_bf")
        nc.vector.tensor_copy(out=b_bf, in_=b)

        # h partial: h[dc] += c_bf[:, ib, dc].T @ b
        # NOTE start=True clears has_written for the WHOLE bank, so only the
        # very first matmul touching this bank may set it.
        for dc in range(NDC):
            nc.tensor.matmul(h_ps[:, dc:dc + 1],
                             lhsT=c_bf[:, ib, dc * P:(dc + 1) * P],
                             rhs=b_bf,
                             start=(ib == 0 and dc == 0),
                             stop=(ib == NIB - 1 and dc == NDC - 1),
                             skip_group_check=True)

    # h AllReduce launches here; it overlaps the U_A phase below.
    h_sb = stat.tile([P, NDC], F32, tag="h_sb")
    nc.scalar.activation(out=h_sb, in_=h_ps, func=AF.Copy)
    hp_dram = dram.tile([D], F32)
    hs_dram = dram.tile([D], F32)
    hp_ap = hp_dram[:]
    nc.sync.dma_start(out=hp_ap.rearrange("(dc p) -> p dc", p=P), in_=h_sb)
    if collective:
        nc.gpsimd.collective_compute(
            "AllReduce", mybir.AluOpType.add,
            replica_groups=[list(range(NCORES))],
            ins=[hp_dram.opt()], outs=[hs_dram.opt()],
        )
    else:
        nc.sync.dma_start(out=hs_dram[:], in_=hp_dram[:])
    hs_ap = hs_dram[:]
    h_bc = consts.tile([P, D], F32)
    nc.sync.dma_start(
        out=h_bc,
        in_=bass.AP(tensor=hs_ap.tensor, offset=hs_ap.offset,
                    ap=[[0, P], [1, D]]),
    )

    # ---- phase 2b: per i-block E.T, U_A, G -------------------------------
    for ib in range(NIB):
        # G block 0 does not depend on anything but the c load
        nc.sync.dma_start(out=g_dram[ib * P:(ib + 1) * P, 0:D], in_=c_nat[ib])

        # E.T via PE transposes; copies on DVE; U_A matmuls follow per group
        et_sb = etpool.tile([P, T], BF16, tag="et")
        ua_ps = uapool.tile([P, D], F32, tag="ua")
        for jg in range(NJS):
            ps = tppool.tile([P, 512], BF16, tag="tp")
            for k in range(4):
                jt = jg * 4 + k
                nc.tensor.transpose(ps[:, k * P:(k + 1) * P],
                                    e_sb[ib][:, jt * P:(jt + 1) * P], ident)
            nc.vector.tensor_copy(out=et_sb[:, jg * 512:(jg + 1) * 512],
                                  in_=ps)
            for k in range(4):
                jc = jg * 4 + k
                nc.tensor.matmul(ua_ps,
                                 lhsT=et_sb[:, jc * P:(jc + 1) * P],
                                 rhs=q_bf[:, jg, k, :],
                                 start=(jc == 0), stop=(jc == NJT - 1))
        ua = gout.tile([P, D], F32, tag="ua_sb")
        nc.scalar.activation(out=ua, in_=ua_ps, func=AF.Copy, scale=zinvs[ib])

        # G blocks 1..2
        nc.sync.dma_start(out=g_dram[ib * P:(ib + 1) * P, D:2 * D], in_=ua)
        cu = gout.tile([P, D], F32, tag="cu")
        nc.vector.tensor_mul(out=cu, in0=c_nat[ib], in1=ua)
        nc.sync.dma_start(out=g_dram[ib * P:(ib + 1) * P, 2 * D:3 * D], in_=cu)

        # G block 3 (c*h) — h_bc arrives while U_A runs
        ch = gout.tile([P, D], F32, tag="ch")
        nc.vector.tensor_mul(out=ch, in0=c_nat[ib], in1=h_bc)
        nc.sync.dma_start(out=g_dram[ib * P:(ib + 1) * P, 3 * D:4 * D], in_=ch)

    ctx.close()


_NC_CACHE = {}


def _get_nc():
    if "nc" not in _NC_CACHE:
        _NC_CACHE["nc"] = build_kernel()
    return _NC_CACHE["nc"]


def kernel(x: np.ndarray, kernel: np.ndarray) -> np.ndarray:
    nc = _get_nc()

    context = np.ascontiguousarray(x[0, 0]).astype(np.float32)   # (T, D)
    question = np.ascontiguousarray(x[1, 0]).astype(np.float32)  # (T, D)
    w = np.asarray(kernel, dtype=np.float32)
    w2 = w[D:2 * D]
    w3 = w[2 * D:3 * D]
    # partition-major chunk layout: wp[p, dc] = w[dc*128 + p]
    w2p = np.ascontiguousarray(w2.reshape(NDC, P).T)
    w3p = np.ascontiguousarray(w3.reshape(NDC, P).T)

    in_maps = []
    for core in range(NCORES):
        in_maps.append({
            "c": np.ascontiguousarray(context[core * TL:(core + 1) * TL]),
            "q": question,
            "w2p": w2p,
            "w3p": w3p,
        })

    res = run_bass_kernel_spmd(nc, in_maps, core_ids=list(range(NCORES)))
    g = np.concatenate([res.results[core]["g"] for core in range(NCORES)],
                       axis=0)
    return g.astype(np.float32)



# revision 4
# speedup vs baseline: 1.2014x; 1.2014x over previous
"""BiAttention (BiDAF-style) kernel for Trainium2, 8 NeuronCores.

Reference math (T=4096, d=512):
    context  = x[0,0]; question = x[1,0]
    S[i,j]   = w1.c_i + w2.q_j + (c_i*w3).q_j
    A        = softmax_j(S)          # w1.c_i is constant per row -> cancels
    U_A      = A @ question
    b        = max_j A[i,j]
    h        = b @ context           # global over T -> one AllReduce
    G        = [context, U_A, context*U_A, context*h]

Sharding: context rows (rows of S/A/U_A/G) split across 8 cores (512 each);
question replicated; h all-reduced (2 KB).

Per-core compute strategy (all big GEMMs in fp8-e4m3 DoubleRow, 4x bf16
PE throughput; S computed TRANSPOSED so exp emits E^T directly and no
E transposes are needed):

  S^T[j,i] = sum_d q[j,d] * (c[i,d]*w3[d] + w2[d])
    - lhsT  = qT (d on partitions), host-pretransposed fp8 hi + lo parts
      (q = q8 + qlo8 error-compensation: halves the fp8 matmul noise,
      needed for the b/h accuracy budget)
    - rhs   = cw3T fp8 = (c^T * w3 + w2), from 16 PE transposes of bf16 c
    - 4 DoubleRow matmuls per 128-row j-tile (2 d-pairs x {hi,lo})
  E^T = exp(S^T - 2) -> bf16 SBUF (ACT; global shift keeps E in fp8 range,
    softmax/max ratios are shift-invariant)
  E8  = fp8(E^T)     (Pool copy; feeds the U_A GEMM)
  Z   = ones @ E8    (DoubleRow ones-matmul, f32 psum)
  b   = pmax(running-max of E^T bf16) / Z   (DVE tensor_max per tile +
        one gpsimd partition_all_reduce; bf16-accurate, fp8-E max would
        blow the tolerance)
  U_A^T[dq,i] = sum_j q8[j,dq] * E8[j,i]  (lhsT = q natural fp8 - no
        transposes; 64 DoubleRow matmuls), then 16 PE transposes back
        and scale by 1/Z.
  h   = b @ c  (tiny bf16 matmuls, f32 psum, 2 KB AllReduce)
  G0  = c exactly via DRAM->DRAM copy; G1..3 written bf16 (within the
        2e-2 budget) and upcast on host.
"""

import numpy as np
import ml_dtypes

import concourse.bass as bass
import concourse.mybir as mybir
import concourse.tile as tile
from concourse import bacc
from concourse import bass_isa
from concourse.bass_utils import run_bass_kernel_spmd
from concourse.masks import make_identity

F32 = mybir.dt.float32
BF16 = mybir.dt.bfloat16
F8 = mybir.dt.float8e4
AF = mybir.ActivationFunctionType

T = 4096
D = 512
NCORES = 8
TL = T // NCORES          # 512 local context rows per core
P = 128
NIB = TL // P             # 4 i-blocks of 128 rows
NJT = T // P              # 32 j-tiles of 128
NDC = D // P              # 4 d-chunks of 128
SHIFT = 2.0               # global logit shift: E = exp(S - 2) <= ~13


def build_kernel(collective=True, compile=True):
    nc = bacc.Bacc("TRN2", target_bir_lowering=False, debug=False,
                   num_devices=NCORES if collective else 1)

    c_dram = nc.dram_tensor("c", [TL, D], F32, kind="ExternalInput").ap()
    q8n_dram = nc.dram_tensor("q8n", [P, NJT, D], F8, kind="ExternalInput").ap()
    qth_dram = nc.dram_tensor("qth", [P, NDC, T], F8, kind="ExternalInput").ap()
    qtl_dram = nc.dram_tensor("qtl", [P, NDC, T], F8, kind="ExternalInput").ap()
    w2p_dram = nc.dram_tensor("w2p", [P, NDC], F32, kind="ExternalInput").ap()
    w3p_dram = nc.dram_tensor("w3p", [P, NDC], F32, kind="ExternalInput").ap()
    g0_dram = nc.dram_tensor("g0", [TL, D], F32, kind="ExternalOutput").ap()
    g123_dram = nc.dram_tensor("g123", [TL, 3 * D], BF16,
                               kind="ExternalOutput").ap()

    with tile.TileContext(nc) as tc:
        _emit(nc, tc, c_dram, q8n_dram, qth_dram, qtl_dram, w2p_dram,
              w3p_dram, g0_dram, g123_dram, collective=collective)

    if compile:
        nc.compile()
    return nc


def _bcast_free(ap_, n):
    """Broadcast a [P, 1] AP along the free axis to [P, n] via 0-stride."""
    return bass.AP(tensor=ap_.tensor, offset=ap_.offset,
                   ap=[ap_.ap[0], [0, n]])


def _row(tile_ap, n):
    """View partition-0 row of a [P, n] tile as a [1, n] AP."""
    return bass.AP(tensor=tile_ap.tensor, offset=tile_ap.offset,
                   ap=[[tile_ap.ap[0][0], 1], [1, n]])


def _emit(nc, tc, c_dram, q8n_dram, qth_dram, qtl_dram, w2p_dram, w3p_dram,
          g0_dram, g123_dram, collective=True):
    from contextlib import ExitStack
    ctx = ExitStack()
    consts = ctx.enter_context(tc.tile_pool(name="consts", bufs=1))
    epool = ctx.enter_context(tc.tile_pool(name="epool", bufs=1))
    gout = ctx.enter_context(tc.tile_pool(name="gout", bufs=2))
    stat = ctx.enter_context(tc.tile_pool(name="stat", bufs=2))
    spool = ctx.enter_context(tc.tile_pool(name="spool", bufs=3, space="PSUM"))
    uapool = ctx.enter_context(tc.tile_pool(name="uapool", bufs=1, space="PSUM"))
    zpool = ctx.enter_context(tc.tile_pool(name="zpool", bufs=1, space="PSUM"))
    dram = ctx.enter_context(tc.tile_pool(name="dram", bufs=1, space="DRAM"))

    # ---- prologue --------------------------------------------------------
    # G0 = context, exact f32, DRAM->DRAM; independent of everything else.
    nc.sync.dma_start(out=g0_dram, in_=c_dram)

    # first qth quarter gates the first S^T matmul - load it before the rest
    qth = consts.tile([P, NDC, T], F8)
    nc.sync.dma_start(out=qth[:, :, 0:1024], in_=qth_dram[:, :, 0:1024])

    # cb cast-load (SWDGE): [p, ib, d] bf16
    cb = consts.tile([P, NIB, D], BF16)
    nc.gpsimd.dma_start(out=cb,
                        in_=c_dram.rearrange("(ib p) d -> p ib d", p=P))

    ident = consts.tile([P, P], BF16)
    make_identity(nc, ident)

    # dummy exp: pull the ACT table load into the startup DMA window
    warm = consts.tile([1, 1], F32)
    nc.vector.memset(warm, 0.0)
    nc.scalar.activation(out=warm, in_=warm, func=AF.Exp)

    ebias = consts.tile([P, 1], F32)
    nc.vector.memset(ebias, -SHIFT)
    ones8 = consts.tile([P, 2, P], F8)
    nc.vector.memset(ones8, 1.0)

    w2p = consts.tile([P, NDC], F32)
    nc.sync.dma_start(out=w2p, in_=w2p_dram)
    w3p = consts.tile([P, NDC], F32)
    nc.sync.dma_start(out=w3p, in_=w3p_dram)

    # HAM warm-up: ramp the PE clock while startup DMAs run
    wa = consts.tile([P, P], BF16)
    nc.vector.memset(wa, 0.0)
    wb = consts.tile([P, 512], BF16)
    nc.vector.memset(wb, 0.0)
    for wi in range(3):
        wps = spool.tile([P, 512], F32, tag="st", name=f"wps{wi}")
        nc.tensor.matmul(wps, lhsT=wa, rhs=wb, start=True, stop=True)

    # remaining loads (DMA serializes; these overlap early compute)
    for qq in range(1, 4):
        nc.sync.dma_start(out=qth[:, :, qq * 1024:(qq + 1) * 1024],
                          in_=qth_dram[:, :, qq * 1024:(qq + 1) * 1024])
    qtl = consts.tile([P, NDC, T], F8)
    for qq in range(4):
        nc.sync.dma_start(out=qtl[:, :, qq * 1024:(qq + 1) * 1024],
                          in_=qtl_dram[:, :, qq * 1024:(qq + 1) * 1024])
    q8n = consts.tile([P, NJT, D], F8)
    for qq in range(4):
        nc.sync.dma_start(out=q8n[:, qq * 8:(qq + 1) * 8, :],
                          in_=q8n_dram[:, qq * 8:(qq + 1) * 8, :])

    # ---- cw3T8[d, i] = c^T * w3 + w2, fp8 --------------------------------
    cw3 = consts.tile([P, NDC, TL], F8)   # [p(d), dc, i]
    for dc in range(NDC):
        ps = spool.tile([P, TL], BF16, tag="st", name=f"tc{dc}")
        for ib in range(NIB):
            nc.tensor.transpose(ps[:, ib * P:(ib + 1) * P],
                                cb[:, ib, dc * P:(dc + 1) * P], ident)
        nc.scalar.activation(out=cw3[:, dc, :], in_=ps, func=AF.Identity,
                             bias=w2p[:, dc:dc + 1], scale=w3p[:, dc:dc + 1])

    # ---- persistent tiles ------------------------------------------------
    e16 = epool.tile([P, NJT, TL], BF16, tag="e16", name="e16")  # [j, jt, i]
    e8 = epool.tile([P, NJT, TL], F8, tag="e8", name="e8")
    macc = stat.tile([P, TL], BF16, tag="macc", name="macc")
    z_ps = zpool.tile([P, TL], F32, tag="z", name="z_ps")
    ua_ps = [uapool.tile([P, TL], F32, tag=f"ua{dqc}", name=f"ua{dqc}")
             for dqc in range(NDC)]

    # ---- main loop over j-tiles -----------------------------------------
    for jt in range(NJT):
        st = spool.tile([P, TL], F32, tag="st", name=f"st{jt}")
        k = 0
        for qt in (qth, qtl):
            for cp in range(2):
                nc.tensor.matmul(
                    st,
                    lhsT=qt[:, 2 * cp:2 * cp + 2, jt * P:(jt + 1) * P],
                    rhs=cw3[:, 2 * cp:2 * cp + 2, :],
                    start=(k == 0), stop=(k == 3),
                    perf_mode=mybir.MatmulPerfMode.DoubleRow)
                k += 1
        nc.scalar.activation(out=e16[:, jt, :], in_=st, func=AF.Exp,
                             bias=ebias, scale=1.0)
        if jt == 0:
            nc.vector.tensor_copy(out=macc, in_=e16[:, jt, :])
        else:
            nc.vector.tensor_max(out=macc, in0=macc, in1=e16[:, jt, :])
        nc.gpsimd.tensor_copy(out=e8[:, jt, :], in_=e16[:, jt, :])

        if jt % 2 == 1:
            tp = jt // 2
            nc.tensor.matmul(z_ps, lhsT=ones8,
                             rhs=e8[:, jt - 1:jt + 1, :],
                             start=(tp == 0), stop=(tp == NJT // 2 - 1),
                             perf_mode=mybir.MatmulPerfMode.DoubleRow,
                             skip_group_check=True)
            for dqc in range(NDC):
                nc.tensor.matmul(
                    ua_ps[dqc],
                    lhsT=q8n[:, jt - 1:jt + 1, dqc * P:(dqc + 1) * P],
                    rhs=e8[:, jt - 1:jt + 1, :],
                    start=(tp == 0), stop=(tp == NJT // 2 - 1),
                    perf_mode=mybir.MatmulPerfMode.DoubleRow,
                    skip_group_check=True)

    # ---- stats: b = pmax(E)/Z -------------------------------------------
    mx = stat.tile([P, TL], F32, tag="mx", name="mx")
    nc.gpsimd.partition_all_reduce(mx, macc, channels=P,
                                   reduce_op=bass_isa.ReduceOp.max)
    mrow_d = dram.tile([TL], F32)
    nc.sync.dma_start(out=mrow_d[:], in_=_row(mx, TL))
    maxe_pp = stat.tile([P, NIB], F32, tag="maxe", name="maxe_pp")
    mr = mrow_d[:]
    nc.sync.dma_start(out=maxe_pp,
                      in_=bass.AP(tensor=mr.tensor, offset=mr.offset,
                                  ap=[[1, P], [P, NIB]]))

    z_sb = stat.tile([1, TL], F32, tag="zsb", name="z_sb")
    nc.vector.tensor_copy(out=z_sb, in_=z_ps[0:1, :])
    zrow_d = dram.tile([TL], F32)
    nc.sync.dma_start(out=zrow_d[:], in_=z_sb[0:1, :])
    z_pp = stat.tile([P, NIB], F32, tag="zpp", name="z_pp")
    zr = zrow_d[:]
    nc.sync.dma_start(out=z_pp,
                      in_=bass.AP(tensor=zr.tensor, offset=zr.offset,
                                  ap=[[1, P], [P, NIB]]))
    zinv = stat.tile([P, NIB], F32, tag="zinv", name="zinv")
    nc.vector.reciprocal(out=zinv, in_=z_pp)

    b_f = stat.tile([P, NIB], F32, tag="bf", name="b_f")
    nc.vector.tensor_mul(out=b_f, in0=maxe_pp, in1=zinv)
    b_bf = stat.tile([P, NIB], BF16, tag="bbf", name="b_bf")
    nc.vector.tensor_copy(out=b_bf, in_=b_f)

    # ---- h partial + AllReduce ------------------------------------------
    h_ps = zpool.tile([P, NDC], F32, tag="z", name="h_ps")
    for ib in range(NIB):
        for dc in range(NDC):
            nc.tensor.matmul(h_ps[:, dc:dc + 1],
                             lhsT=cb[:, ib, dc * P:(dc + 1) * P],
                             rhs=b_bf[:, ib:ib + 1],
                             start=(ib == 0 and dc == 0),
                             stop=(ib == NIB - 1 and dc == NDC - 1),
                             skip_group_check=True)
    h_sb = stat.tile([P, NDC], F32, tag="hsb", name="h_sb")
    nc.scalar.activation(out=h_sb, in_=h_ps, func=AF.Copy)
    hp_dram = dram.tile([D], F32)
    hs_dram = dram.tile([D], F32)
    hp_ap = hp_dram[:]
    nc.sync.dma_start(out=hp_ap.rearrange("(dc p) -> p dc", p=P), in_=h_sb)
    if collective:
        nc.gpsimd.collective_compute(
            "AllReduce", mybir.AluOpType.add,
            replica_groups=[list(range(NCORES))],
            ins=[hp_dram.opt()], outs=[hs_dram.opt()],
        )
    else:
        nc.sync.dma_start(out=hs_dram[:], in_=hp_dram[:])
    hs_ap = hs_dram[:]
    h_bc = consts.tile([P, D], F32)
    nc.sync.dma_start(
        out=h_bc,
        in_=bass.AP(tensor=hs_ap.tensor, offset=hs_ap.offset,
                    ap=[[0, P], [1, D]]),
    )

    # ---- U_A^T -> U_A, G1..3 --------------------------------------------
    uat = consts.tile([P, NDC, TL], BF16)   # [p(dq), dqc, i]
    for dqc in range(NDC):
        nc.vector.tensor_copy(out=uat[:, dqc, :], in_=ua_ps[dqc])

    for ib in range(NIB):
        ps = spool.tile([P, D], BF16, tag="st", name=f"uat{ib}")
        for dqc in range(NDC):
            nc.tensor.transpose(ps[:, dqc * P:(dqc + 1) * P],
                                uat[:, dqc, ib * P:(ib + 1) * P], ident)
        g = gout.tile([P, 3, D], BF16, tag="g", name=f"g{ib}")
        # G1 = U_A = U_A^T.T * zinv
        nc.scalar.activation(out=g[:, 0, :], in_=ps, func=AF.Copy,
                             scale=zinv[:, ib:ib + 1])
        # G2 = c * U_A
        nc.vector.tensor_mul(out=g[:, 1, :], in0=g[:, 0, :], in1=cb[:, ib, :])
        # G3 = c * h
        nc.vector.tensor_mul(out=g[:, 2, :], in0=cb[:, ib, :], in1=h_bc)
        nc.sync.dma_start(out=g123_dram[ib * P:(ib + 1) * P, :], in_=g)

    ctx.close()


_NC_CACHE = {}


def _get_nc():
    if "nc" not in _NC_CACHE:
        _NC_CACHE["nc"] = build_kernel()
    return _NC_CACHE["nc"]


def _prep_inputs(x, kernel):
    """Host-side layout prep shared by kernel() and test harnesses."""
    context = np.ascontiguousarray(x[0, 0]).astype(np.float32)   # (T, D)
    question = np.ascontiguousarray(x[1, 0]).astype(np.float32)  # (T, D)
    w = np.asarray(kernel, dtype=np.float32)
    w2 = w[D:2 * D]
    w3 = w[2 * D:3 * D]
    w2p = np.ascontiguousarray(w2.reshape(NDC, P).T)
    w3p = np.ascontiguousarray(w3.reshape(NDC, P).T)

    q8 = question.astype(ml_dtypes.float8_e4m3)
    qlo8 = (question - q8.astype(np.float32)).astype(ml_dtypes.float8_e4m3)
    # q8n[p, jt, dq] = q8[jt*128 + p, dq]
    q8n = np.ascontiguousarray(q8.reshape(NJT, P, D).transpose(1, 0, 2))
    # qth[p, dc, j] = q8[j, dc*128 + p]
    qth = np.ascontiguousarray(q8.T.reshape(NDC, P, T).transpose(1, 0, 2))
    qtl = np.ascontiguousarray(qlo8.T.reshape(NDC, P, T).transpose(1, 0, 2))

    shared = {"q8n": q8n, "qth": qth, "qtl": qtl, "w2p": w2p, "w3p": w3p}
    in_maps = []
    for core in range(NCORES):
        m = dict(shared)
        m["c"] = np.ascontiguousarray(context[core * TL:(core + 1) * TL])
        in_maps.append(m)
    return in_maps


def _assemble(results):
    out = []
    for core in range(NCORES):
        g0 = np.asarray(results[core]["g0"], dtype=np.float32)
        g123 = np.asarray(results[core]["g123"]).astype(np.float32)
        out.append(np.concatenate([g0, g123], axis=1))
    return np.concatenate(out, axis=0)


def kernel(x: np.ndarray, kernel: np.ndarray) -> np.ndarray:
    nc = _get_nc()
    in_maps = _prep_inputs(x, kernel)
    res = run_bass_kernel_spmd(nc, in_maps, core_ids=list(range(NCORES)))
    return _assemble(res.results).astype(np.float32)


# revision 5
# speedup vs baseline: 1.4699x; 1.2235x over previous
"""BiAttention (BiDAF-style) kernel for Trainium2, 8 NeuronCores.

Reference math (T=4096, d=512):
    context  = x[0,0]; question = x[1,0]
    S[i,j]   = w1.c_i + w2.q_j + (c_i*w3).q_j
    A        = softmax_j(S)          # w1.c_i is constant per row -> cancels
    U_A      = A @ question
    b        = max_j A[i,j]
    h        = b @ context           # global over T -> one AllReduce
    G        = [context, U_A, context*U_A, context*h]

Sharding: context rows (rows of S/A/U_A/G) split across 8 cores (512 each);
question replicated; h all-reduced (2 KB).

Per-core compute strategy (all big GEMMs in fp8-e4m3 DoubleRow, 4x bf16
PE throughput; S computed TRANSPOSED so exp emits E^T directly and no
E transposes are needed):

  S^T[j,i] = sum_d q[j,d] * (c[i,d]*w3[d] + w2[d])
    - lhsT  = qT (d on partitions), host-pretransposed fp8 hi + lo parts
      (q = q8 + qlo8 error-compensation: halves the fp8 matmul noise,
      needed for the b/h accuracy budget)
    - rhs   = cw3T fp8 = (c^T * w3 + w2), from 16 PE transposes of bf16 c
    - 4 DoubleRow matmuls per 128-row j-tile (2 d-pairs x {hi,lo})
  E^T = exp(S^T - 2) -> bf16 SBUF (ACT; global shift keeps E in fp8 range,
    softmax/max ratios are shift-invariant)
  E8  = fp8(E^T)     (Pool copy; feeds the U_A GEMM)
  Z   = ones @ E8    (DoubleRow ones-matmul, f32 psum)
  b   = pmax(running-max of E^T bf16) / Z   (DVE tensor_max per tile +
        one gpsimd partition_all_reduce; bf16-accurate, fp8-E max would
        blow the tolerance)
  U_A^T[dq,i] = sum_j q8[j,dq] * E8[j,i]  (lhsT = q natural fp8 - no
        transposes; 64 DoubleRow matmuls), then 16 PE transposes back
        and scale by 1/Z.
  h   = b @ c  (tiny bf16 matmuls, f32 psum, 2 KB AllReduce)
  G0  = c exactly via DRAM->DRAM copy; G1..3 written bf16 (within the
        2e-2 budget) and upcast on host.
"""

import numpy as np
import ml_dtypes

import concourse.bass as bass
import concourse.mybir as mybir
import concourse.tile as tile
from concourse import bacc
from concourse import bass_isa
from concourse.bass_utils import run_bass_kernel_spmd
from concourse.masks import make_identity

F32 = mybir.dt.float32
BF16 = mybir.dt.bfloat16
F8 = mybir.dt.float8e4
AF = mybir.ActivationFunctionType

T = 4096
D = 512
NCORES = 8
TL = T // NCORES          # 512 local context rows per core
P = 128
NIB = TL // P             # 4 i-blocks of 128 rows
NJT = T // P              # 32 j-tiles of 128
NDC = D // P              # 4 d-chunks of 128
SHIFT = 2.0               # global logit shift: E = exp(S - 2) <= ~13


def build_kernel(collective=True, compile=True):
    nc = bacc.Bacc("TRN2", target_bir_lowering=False, debug=False,
                   num_devices=NCORES if collective else 1)

    c_dram = nc.dram_tensor("c", [TL, D], F32, kind="ExternalInput").ap()
    q8n_dram = nc.dram_tensor("q8n", [P, NJT, D], F8, kind="ExternalInput").ap()
    qth_dram = nc.dram_tensor("qth", [P, NDC, T], F8, kind="ExternalInput").ap()
    qtl_dram = nc.dram_tensor("qtl", [P, NDC, T], F8, kind="ExternalInput").ap()
    w2p_dram = nc.dram_tensor("w2p", [P, NDC], F32, kind="ExternalInput").ap()
    w3p_dram = nc.dram_tensor("w3p", [P, NDC], F32, kind="ExternalInput").ap()
    g0_dram = nc.dram_tensor("g0", [TL, D], F32, kind="ExternalOutput").ap()
    g123_dram = nc.dram_tensor("g123", [TL, 3 * D], BF16,
                               kind="ExternalOutput").ap()

    with tile.TileContext(nc) as tc:
        _emit(nc, tc, c_dram, q8n_dram, qth_dram, qtl_dram, w2p_dram,
              w3p_dram, g0_dram, g123_dram, collective=collective)

    if compile:
        nc.compile()
    return nc


def _bcast_free(ap_, n):
    """Broadcast a [P, 1] AP along the free axis to [P, n] via 0-stride."""
    return bass.AP(tensor=ap_.tensor, offset=ap_.offset,
                   ap=[ap_.ap[0], [0, n]])


def _row(tile_ap, n):
    """View partition-0 row of a [P, n] tile as a [1, n] AP."""
    return bass.AP(tensor=tile_ap.tensor, offset=tile_ap.offset,
                   ap=[[tile_ap.ap[0][0], 1], [1, n]])


def _emit(nc, tc, c_dram, q8n_dram, qth_dram, qtl_dram, w2p_dram, w3p_dram,
          g0_dram, g123_dram, collective=True):
    from contextlib import ExitStack
    ctx = ExitStack()
    consts = ctx.enter_context(tc.tile_pool(name="consts", bufs=1))
    epool = ctx.enter_context(tc.tile_pool(name="epool", bufs=1))
    gout = ctx.enter_context(tc.tile_pool(name="gout", bufs=2))
    stat = ctx.enter_context(tc.tile_pool(name="stat", bufs=2))
    spool = ctx.enter_context(tc.tile_pool(name="spool", bufs=3, space="PSUM"))
    uapool = ctx.enter_context(tc.tile_pool(name="uapool", bufs=1, space="PSUM"))
    zpool = ctx.enter_context(tc.tile_pool(name="zpool", bufs=1, space="PSUM"))
    dram = ctx.enter_context(tc.tile_pool(name="dram", bufs=1, space="DRAM"))

    # ---- prologue --------------------------------------------------------
    # cb cast-load (SWDGE): [p, ib, d] bf16; gates cw3T which gates all S^T
    cb = consts.tile([P, NIB, D], BF16)
    nc.gpsimd.dma_start(out=cb,
                        in_=c_dram.rearrange("(ib p) d -> p ib d", p=P))

    ident = consts.tile([P, P], BF16)
    make_identity(nc, ident)

    # dummy exp: pull the ACT table load into the startup DMA window
    warm = consts.tile([1, 1], F32)
    nc.vector.memset(warm, 0.0)
    nc.scalar.activation(out=warm, in_=warm, func=AF.Exp)

    ebias = consts.tile([P, 1], F32)
    nc.vector.memset(ebias, -SHIFT)
    ones8 = consts.tile([P, 2, P], F8)
    nc.vector.memset(ones8, 1.0)

    w2p = consts.tile([P, NDC], F32)
    nc.sync.dma_start(out=w2p, in_=w2p_dram)
    w3p = consts.tile([P, NDC], F32)
    nc.sync.dma_start(out=w3p, in_=w3p_dram)

    # HAM warm-up: ramp the PE clock while startup DMAs run
    wa = consts.tile([P, P], BF16)
    nc.vector.memset(wa, 0.0)
    wb = consts.tile([P, 512], BF16)
    nc.vector.memset(wb, 0.0)
    for wi in range(3):
        wps = spool.tile([P, 512], F32, tag="st", name=f"wps{wi}")
        nc.tensor.matmul(wps, lhsT=wa, rhs=wb, start=True, stop=True)

    # q loads: 8 interleaved groups of (qth, qtl, q8n) 4-tile slices so the
    # S^T/UA pipeline starts after ~3 small DMAs instead of after all of q
    qth = consts.tile([P, NDC, T], F8)
    qtl = consts.tile([P, NDC, T], F8)
    q8n = consts.tile([P, NJT, D], F8)
    for g in range(8):
        j0, j1 = g * 512, (g + 1) * 512
        nc.sync.dma_start(out=qth[:, :, j0:j1], in_=qth_dram[:, :, j0:j1])
        nc.sync.dma_start(out=qtl[:, :, j0:j1], in_=qtl_dram[:, :, j0:j1])
        nc.sync.dma_start(out=q8n[:, g * 4:(g + 1) * 4, :],
                          in_=q8n_dram[:, g * 4:(g + 1) * 4, :])

    # ---- cw3T8[d, i] = c^T * w3 + w2, fp8 --------------------------------
    cw3 = consts.tile([P, NDC, TL], F8)   # [p(d), dc, i]
    for dc in range(NDC):
        ps = spool.tile([P, TL], BF16, tag="st", name=f"tc{dc}")
        for ib in range(NIB):
            nc.tensor.transpose(ps[:, ib * P:(ib + 1) * P],
                                cb[:, ib, dc * P:(dc + 1) * P], ident)
        nc.scalar.activation(out=cw3[:, dc, :], in_=ps, func=AF.Identity,
                             bias=w2p[:, dc:dc + 1], scale=w3p[:, dc:dc + 1])

    # ---- persistent tiles ------------------------------------------------
    e16 = epool.tile([P, NJT, TL], BF16, tag="e16", name="e16")  # [j, jt, i]
    e8 = epool.tile([P, NJT, TL], F8, tag="e8", name="e8")
    macc = stat.tile([P, TL], BF16, tag="macc", name="macc")
    z_ps = zpool.tile([P, TL], F32, tag="z", name="z_ps")
    ua_ps = [uapool.tile([P, TL], F32, tag=f"ua{dqc}", name=f"ua{dqc}")
             for dqc in range(NDC)]

    # ---- main loop over j-tiles -----------------------------------------
    for jt in range(NJT):
        st = spool.tile([P, TL], F32, tag="st", name=f"st{jt}")
        k = 0
        for qt in (qth, qtl):
            for cp in range(2):
                nc.tensor.matmul(
                    st,
                    lhsT=qt[:, 2 * cp:2 * cp + 2, jt * P:(jt + 1) * P],
                    rhs=cw3[:, 2 * cp:2 * cp + 2, :],
                    start=(k == 0), stop=(k == 3),
                    perf_mode=mybir.MatmulPerfMode.DoubleRow)
                k += 1
        nc.scalar.activation(out=e16[:, jt, :], in_=st, func=AF.Exp,
                             bias=ebias, scale=1.0)
        if jt == 0:
            nc.vector.tensor_copy(out=macc, in_=e16[:, jt, :])
        else:
            nc.vector.tensor_max(out=macc, in0=macc, in1=e16[:, jt, :])
        cast_eng = nc.gpsimd if jt % 2 == 0 else nc.vector
        cast_eng.tensor_copy(out=e8[:, jt, :], in_=e16[:, jt, :])

        if jt % 2 == 1:
            tp = jt // 2
            nc.tensor.matmul(z_ps, lhsT=ones8,
                             rhs=e8[:, jt - 1:jt + 1, :],
                             start=(tp == 0), stop=(tp == NJT // 2 - 1),
                             perf_mode=mybir.MatmulPerfMode.DoubleRow,
                             skip_group_check=True)
            for dqc in range(NDC):
                nc.tensor.matmul(
                    ua_ps[dqc],
                    lhsT=q8n[:, jt - 1:jt + 1, dqc * P:(dqc + 1) * P],
                    rhs=e8[:, jt - 1:jt + 1, :],
                    start=(tp == 0), stop=(tp == NJT // 2 - 1),
                    perf_mode=mybir.MatmulPerfMode.DoubleRow,
                    skip_group_check=True)

    # G0 = context, exact f32, DRAM->DRAM (independent; emitted late so the
    # startup DMAs feed the matmul pipeline first)
    nc.sync.dma_start(out=g0_dram, in_=c_dram)

    # ---- stats: b = pmax(E)/Z -------------------------------------------
    mx = stat.tile([P, TL], F32, tag="mx", name="mx")
    nc.gpsimd.partition_all_reduce(mx, macc, channels=P,
                                   reduce_op=bass_isa.ReduceOp.max)
    mrow_d = dram.tile([TL], F32)
    nc.sync.dma_start(out=mrow_d[:], in_=_row(mx, TL))
    maxe_pp = stat.tile([P, NIB], F32, tag="maxe", name="maxe_pp")
    mr = mrow_d[:]
    nc.sync.dma_start(out=maxe_pp,
                      in_=bass.AP(tensor=mr.tensor, offset=mr.offset,
                                  ap=[[1, P], [P, NIB]]))

    z_sb = stat.tile([1, TL], F32, tag="zsb", name="z_sb")
    nc.vector.tensor_copy(out=z_sb, in_=z_ps[0:1, :])
    zrow_d = dram.tile([TL], F32)
    nc.sync.dma_start(out=zrow_d[:], in_=z_sb[0:1, :])
    z_pp = stat.tile([P, NIB], F32, tag="zpp", name="z_pp")
    zr = zrow_d[:]
    nc.sync.dma_start(out=z_pp,
                      in_=bass.AP(tensor=zr.tensor, offset=zr.offset,
                                  ap=[[1, P], [P, NIB]]))
    zinv = stat.tile([P, NIB], F32, tag="zinv", name="zinv")
    nc.vector.reciprocal(out=zinv, in_=z_pp)

    b_f = stat.tile([P, NIB], F32, tag="bf", name="b_f")
    nc.vector.tensor_mul(out=b_f, in0=maxe_pp, in1=zinv)
    b_bf = stat.tile([P, NIB], BF16, tag="bbf", name="b_bf")
    nc.vector.tensor_copy(out=b_bf, in_=b_f)

    # ---- h partial + AllReduce ------------------------------------------
    h_ps = zpool.tile([P, NDC], F32, tag="z", name="h_ps")
    for ib in range(NIB):
        for dc in range(NDC):
            nc.tensor.matmul(h_ps[:, dc:dc + 1],
                             lhsT=cb[:, ib, dc * P:(dc + 1) * P],
                             rhs=b_bf[:, ib:ib + 1],
                             start=(ib == 0 and dc == 0),
                             stop=(ib == NIB - 1 and dc == NDC - 1),
                             skip_group_check=True)
    h_sb = stat.tile([P, NDC], F32, tag="hsb", name="h_sb")
    nc.scalar.activation(out=h_sb, in_=h_ps, func=AF.Copy)
    hp_dram = dram.tile([D], F32)
    hs_dram = dram.tile([D], F32)
    hp_ap = hp_dram[:]
    nc.sync.dma_start(out=hp_ap.rearrange("(dc p) -> p dc", p=P), in_=h_sb)
    if collective:
        nc.gpsimd.collective_compute(
            "AllReduce", mybir.AluOpType.add,
            replica_groups=[list(range(NCORES))],
            ins=[hp_dram.opt()], outs=[hs_dram.opt()],
        )
    else:
        nc.sync.dma_start(out=hs_dram[:], in_=hp_dram[:])
    hs_ap = hs_dram[:]
    h_bc = consts.tile([P, D], F32)
    nc.sync.dma_start(
        out=h_bc,
        in_=bass.AP(tensor=hs_ap.tensor, offset=hs_ap.offset,
                    ap=[[0, P], [1, D]]),
    )

    # ---- U_A^T -> U_A, G1..3 --------------------------------------------
    uat = consts.tile([P, NDC, TL], BF16)   # [p(dq), dqc, i]
    for dqc in range(NDC):
        nc.vector.tensor_copy(out=uat[:, dqc, :], in_=ua_ps[dqc])

    for ib in range(NIB):
        ps = spool.tile([P, D], BF16, tag="st", name=f"uat{ib}")
        for dqc in range(NDC):
            nc.tensor.transpose(ps[:, dqc * P:(dqc + 1) * P],
                                uat[:, dqc, ib * P:(ib + 1) * P], ident)
        g = gout.tile([P, 3, D], BF16, tag="g", name=f"g{ib}")
        # G1 = U_A = U_A^T.T * zinv
        nc.scalar.activation(out=g[:, 0, :], in_=ps, func=AF.Copy,
                             scale=zinv[:, ib:ib + 1])
        # G2 = c * U_A
        nc.vector.tensor_mul(out=g[:, 1, :], in0=g[:, 0, :], in1=cb[:, ib, :])
        # G3 = c * h
        nc.vector.tensor_mul(out=g[:, 2, :], in0=cb[:, ib, :], in1=h_bc)
        nc.sync.dma_start(out=g123_dram[ib * P:(ib + 1) * P, :], in_=g)

    ctx.close()


_NC_CACHE = {}


def _get_nc():
    if "nc" not in _NC_CACHE:
        _NC_CACHE["nc"] = build_kernel()
    return _NC_CACHE["nc"]


def _prep_inputs(x, kernel):
    """Host-side layout prep shared by kernel() and test harnesses."""
    context = np.ascontiguousarray(x[0, 0]).astype(np.float32)   # (T, D)
    question = np.ascontiguousarray(x[1, 0]).astype(np.float32)  # (T, D)
    w = np.asarray(kernel, dtype=np.float32)
    w2 = w[D:2 * D]
    w3 = w[2 * D:3 * D]
    w2p = np.ascontiguousarray(w2.reshape(NDC, P).T)
    w3p = np.ascontiguousarray(w3.reshape(NDC, P).T)

    q8 = question.astype(ml_dtypes.float8_e4m3)
    qlo8 = (question - q8.astype(np.float32)).astype(ml_dtypes.float8_e4m3)
    # q8n[p, jt, dq] = q8[jt*128 + p, dq]
    q8n = np.ascontiguousarray(q8.reshape(NJT, P, D).transpose(1, 0, 2))
    # qth[p, dc, j] = q8[j, dc*128 + p]
    qth = np.ascontiguousarray(q8.T.reshape(NDC, P, T).transpose(1, 0, 2))
    qtl = np.ascontiguousarray(qlo8.T.reshape(NDC, P, T).transpose(1, 0, 2))

    shared = {"q8n": q8n, "qth": qth, "qtl": qtl, "w2p": w2p, "w3p": w3p}
    in_maps = []
    for core in range(NCORES):
        m = dict(shared)
        m["c"] = np.ascontiguousarray(context[core * TL:(core + 1) * TL])
        in_maps.append(m)
    return in_maps


def _assemble(results):
    out = []
    for core in range(NCORES):
        g0 = np.asarray(results[core]["g0"], dtype=np.float32)
        g123 = np.asarray(results[core]["g123"]).astype(np.float32)
        out.append(np.concatenate([g0, g123], axis=1))
    return np.concatenate(out, axis=0)


def kernel(x: np.ndarray, kernel: np.ndarray) -> np.ndarray:
    nc = _get_nc()
    in_maps = _prep_inputs(x, kernel)
    res = run_bass_kernel_spmd(nc, in_maps, core_ids=list(range(NCORES)))
    return _assemble(res.results).astype(np.float32)


# revision 7
# speedup vs baseline: 1.5294x; 1.0405x over previous
"""BiAttention (BiDAF-style) kernel for Trainium2, 8 NeuronCores.

Reference math (T=4096, d=512):
    context  = x[0,0]; question = x[1,0]
    S[i,j]   = w1.c_i + w2.q_j + (c_i*w3).q_j
    A        = softmax_j(S)          # w1.c_i is constant per row -> cancels
    U_A      = A @ question
    b        = max_j A[i,j]
    h        = b @ context           # global over T -> one AllReduce
    G        = [context, U_A, context*U_A, context*h]

Sharding: context rows (rows of S/A/U_A/G) split across 8 cores (512 each);
question replicated; h all-reduced (2 KB).

Per-core compute strategy (all big GEMMs in fp8-e4m3 DoubleRow, 4x bf16
PE throughput; S computed TRANSPOSED so exp emits E^T directly and no
E transposes are needed):

  S^T[j,i] = sum_d q[j,d] * (c[i,d]*w3[d] + w2[d])
    - lhsT  = qT (d on partitions), host-pretransposed fp8 hi + lo parts
      (q = q8 + qlo8 error-compensation: halves the fp8 matmul noise,
      needed for the b/h accuracy budget)
    - rhs   = cw3T fp8 = (c^T * w3 + w2), from 16 PE transposes of bf16 c
    - 4 DoubleRow matmuls per 128-row j-tile (2 d-pairs x {hi,lo})
  E^T = exp(S^T - 2) -> bf16 SBUF (ACT; global shift keeps E in fp8 range,
    softmax/max ratios are shift-invariant)
  E8  = fp8(E^T)     (Pool copy; feeds the U_A GEMM)
  Z   = ones @ E8    (DoubleRow ones-matmul, f32 psum)
  b   = pmax(running-max of E^T bf16) / Z   (DVE tensor_max per tile +
        one gpsimd partition_all_reduce; bf16-accurate, fp8-E max would
        blow the tolerance)
  U_A^T[dq,i] = sum_j q8[j,dq] * E8[j,i]  (lhsT = q natural fp8 - no
        transposes; 64 DoubleRow matmuls), then 16 PE transposes back
        and scale by 1/Z.
  h   = b @ c  (tiny bf16 matmuls, f32 psum, 2 KB AllReduce)
  G0  = c exactly via DRAM->DRAM copy; G1..3 written bf16 (within the
        2e-2 budget) and upcast on host.
"""

import numpy as np
import ml_dtypes

import concourse.bass as bass
import concourse.mybir as mybir
import concourse.tile as tile
from concourse import bacc
from concourse import bass_isa
from concourse.bass_utils import run_bass_kernel_spmd
from concourse.masks import make_identity

F32 = mybir.dt.float32
BF16 = mybir.dt.bfloat16
F8 = mybir.dt.float8e4
AF = mybir.ActivationFunctionType

T = 4096
D = 512
NCORES = 8
TL = T // NCORES          # 512 local context rows per core
P = 128
NIB = TL // P             # 4 i-blocks of 128 rows
NJT = T // P              # 32 j-tiles of 128
NDC = D // P              # 4 d-chunks of 128
SHIFT = 2.0               # global logit shift: E = exp(S - 2) <= ~13


def build_kernel(collective=True, compile=True):
    nc = bacc.Bacc("TRN2", target_bir_lowering=False, debug=False,
                   num_devices=NCORES if collective else 1)

    c_dram = nc.dram_tensor("c", [TL, D], F32, kind="ExternalInput").ap()
    q8n_dram = nc.dram_tensor("q8n", [P, NJT, D], F8, kind="ExternalInput").ap()
    qth_dram = nc.dram_tensor("qth", [P, NDC, T], F8, kind="ExternalInput").ap()
    qtl_dram = nc.dram_tensor("qtl", [P, NDC, T], F8, kind="ExternalInput").ap()
    w2p_dram = nc.dram_tensor("w2p", [P, NDC], F32, kind="ExternalInput").ap()
    w3p_dram = nc.dram_tensor("w3p", [P, NDC], F32, kind="ExternalInput").ap()
    g0_dram = nc.dram_tensor("g0", [TL, D], F32, kind="ExternalOutput").ap()
    g123_dram = nc.dram_tensor("g123", [TL, 3 * D], BF16,
                               kind="ExternalOutput").ap()

    with tile.TileContext(nc) as tc:
        _emit(nc, tc, c_dram, q8n_dram, qth_dram, qtl_dram, w2p_dram,
              w3p_dram, g0_dram, g123_dram, collective=collective)

    if compile:
        nc.compile()
    return nc


def _bcast_free(ap_, n):
    """Broadcast a [P, 1] AP along the free axis to [P, n] via 0-stride."""
    return bass.AP(tensor=ap_.tensor, offset=ap_.offset,
                   ap=[ap_.ap[0], [0, n]])


def _row(tile_ap, n):
    """View partition-0 row of a [P, n] tile as a [1, n] AP."""
    return bass.AP(tensor=tile_ap.tensor, offset=tile_ap.offset,
                   ap=[[tile_ap.ap[0][0], 1], [1, n]])


def _emit(nc, tc, c_dram, q8n_dram, qth_dram, qtl_dram, w2p_dram, w3p_dram,
          g0_dram, g123_dram, collective=True):
    from contextlib import ExitStack
    ctx = ExitStack()
    consts = ctx.enter_context(tc.tile_pool(name="consts", bufs=1))
    epool = ctx.enter_context(tc.tile_pool(name="epool", bufs=1))
    gout = ctx.enter_context(tc.tile_pool(name="gout", bufs=2))
    stat = ctx.enter_context(tc.tile_pool(name="stat", bufs=2))
    spool = ctx.enter_context(tc.tile_pool(name="spool", bufs=3, space="PSUM"))
    uapool = ctx.enter_context(tc.tile_pool(name="uapool", bufs=1, space="PSUM"))
    zpool = ctx.enter_context(tc.tile_pool(name="zpool", bufs=1, space="PSUM"))
    dram = ctx.enter_context(tc.tile_pool(name="dram", bufs=1, space="DRAM"))

    # ---- prologue --------------------------------------------------------
    # cb cast-load (SWDGE): [p, ib, d] bf16; gates cw3T which gates all S^T
    cb = consts.tile([P, NIB, D], BF16)
    nc.gpsimd.dma_start(out=cb,
                        in_=c_dram.rearrange("(ib p) d -> p ib d", p=P))

    ident = consts.tile([P, P], BF16)
    make_identity(nc, ident)

    # dummy exp: pull the ACT table load into the startup DMA window
    warm = consts.tile([1, 1], F32)
    nc.vector.memset(warm, 0.0)
    nc.scalar.activation(out=warm, in_=warm, func=AF.Exp)

    ebias = consts.tile([P, 1], F32)
    nc.vector.memset(ebias, -SHIFT)
    ones8 = consts.tile([P, 2, 1], F8)
    nc.vector.memset(ones8, 1.0)

    w2p = consts.tile([P, NDC], F32)
    nc.sync.dma_start(out=w2p, in_=w2p_dram)
    w3p = consts.tile([P, NDC], F32)
    nc.sync.dma_start(out=w3p, in_=w3p_dram)

    # HAM warm-up: ramp the PE clock while startup DMAs run
    wa = consts.tile([P, P], BF16)
    nc.vector.memset(wa, 0.0)
    wb = consts.tile([P, 512], BF16)
    nc.vector.memset(wb, 0.0)
    for wi in range(3):
        wps = spool.tile([P, 512], F32, tag="st", name=f"wps{wi}")
        nc.tensor.matmul(wps, lhsT=wa, rhs=wb, start=True, stop=True)

    # q loads: 8 interleaved groups of (qth, qtl, q8n) 4-tile slices so the
    # S^T/UA pipeline starts after ~3 small DMAs instead of after all of q
    qth = consts.tile([P, NDC, T], F8)
    qtl = consts.tile([P, NDC, T], F8)
    q8n = consts.tile([P, NJT, D], F8)
    for g in range(8):
        j0, j1 = g * 512, (g + 1) * 512
        nc.sync.dma_start(out=qth[:, :, j0:j1], in_=qth_dram[:, :, j0:j1])
        nc.sync.dma_start(out=qtl[:, :, j0:j1], in_=qtl_dram[:, :, j0:j1])
        nc.sync.dma_start(out=q8n[:, g * 4:(g + 1) * 4, :],
                          in_=q8n_dram[:, g * 4:(g + 1) * 4, :])

    # ---- cw3T8[d, i] = c^T * w3 + w2, fp8 --------------------------------
    cw3 = consts.tile([P, NDC, TL], F8)   # [p(d), dc, i]
    for dc in range(NDC):
        ps = spool.tile([P, TL], BF16, tag="st", name=f"tc{dc}")
        for ib in range(NIB):
            nc.tensor.transpose(ps[:, ib * P:(ib + 1) * P],
                                cb[:, ib, dc * P:(dc + 1) * P], ident)
        nc.scalar.activation(out=cw3[:, dc, :], in_=ps, func=AF.Identity,
                             bias=w2p[:, dc:dc + 1], scale=w3p[:, dc:dc + 1])

    # ---- persistent tiles ------------------------------------------------
    e16 = epool.tile([P, NJT, TL], BF16, tag="e16", name="e16")  # [j, jt, i]
    e8 = epool.tile([P, NJT, TL], F8, tag="e8", name="e8")
    macc = stat.tile([P, TL], BF16, tag="macc", name="macc")
    z2 = zpool.tile([P, NIB], F32, tag="z", name="z2")
    ua_ps = [uapool.tile([P, TL], F32, tag=f"ua{dqc}", name=f"ua{dqc}")
             for dqc in range(NDC)]

    # ---- main loop over j-tiles -----------------------------------------
    def emit_pair(tp):
        """Z + U_A DoubleRow matmuls for j-tile pair tp (tiles 2tp, 2tp+1)."""
        jt0 = 2 * tp
        for ib in range(NIB):
            # start=True clears the WHOLE bank: only the first matmul
            # touching the z2 bank may set it
            nc.tensor.matmul(z2[:, ib:ib + 1],
                             lhsT=e8[:, jt0:jt0 + 2, ib * P:(ib + 1) * P],
                             rhs=ones8,
                             start=(tp == 0 and ib == 0),
                             stop=(tp == NJT // 2 - 1),
                             perf_mode=mybir.MatmulPerfMode.DoubleRow,
                             skip_group_check=True)
        for dqc in range(NDC):
            nc.tensor.matmul(
                ua_ps[dqc],
                lhsT=q8n[:, jt0:jt0 + 2, dqc * P:(dqc + 1) * P],
                rhs=e8[:, jt0:jt0 + 2, :],
                start=(tp == 0), stop=(tp == NJT // 2 - 1),
                perf_mode=mybir.MatmulPerfMode.DoubleRow,
                skip_group_check=True)

    for jt in range(NJT):
        st = spool.tile([P, TL], F32, tag="st", name=f"st{jt}")
        k = 0
        for qt in (qth, qtl):
            for cp in range(2):
                nc.tensor.matmul(
                    st,
                    lhsT=qt[:, 2 * cp:2 * cp + 2, jt * P:(jt + 1) * P],
                    rhs=cw3[:, 2 * cp:2 * cp + 2, :],
                    start=(k == 0), stop=(k == 3),
                    perf_mode=mybir.MatmulPerfMode.DoubleRow)
                k += 1
        nc.scalar.activation(out=e16[:, jt, :], in_=st, func=AF.Exp,
                             bias=ebias, scale=1.0)
        if jt == 0:
            nc.vector.tensor_copy(out=macc, in_=e16[:, jt, :])
        else:
            nc.vector.tensor_max(out=macc, in0=macc, in1=e16[:, jt, :])
        cast_eng = nc.gpsimd if jt % 8 < 5 else nc.vector
        cast_eng.tensor_copy(out=e8[:, jt, :], in_=e16[:, jt, :])
        # pair 15 is deferred to after the b/h launch to fill the AllReduce
        # latency window with U_A work
        if jt % 2 == 1 and jt != NJT - 1:
            emit_pair(jt // 2)

    # ---- stats: maxe via PE transpose of the running max (partition max
    # without gpsimd: transpose [j-lane, i] -> [i, j-lane], then a free-axis
    # DVE max directly in the [p, ib] layout the h-matmul needs)
    maccT = spool.tile([P, NIB, P], BF16, tag="st", name="maccT")
    for ib in range(NIB):
        nc.tensor.transpose(maccT[:, ib, :],
                            macc[:, ib * P:(ib + 1) * P], ident)
    emit_pair(NJT // 2 - 1)

    maxe_pp = stat.tile([P, NIB], F32, tag="maxe", name="maxe_pp")
    nc.vector.tensor_reduce(out=maxe_pp, in_=maccT,
                            axis=mybir.AxisListType.X,
                            op=mybir.AluOpType.max)
    zinv = stat.tile([P, NIB], F32, tag="zinv", name="zinv")
    nc.vector.reciprocal(out=zinv, in_=z2)
    b_f = stat.tile([P, NIB], F32, tag="bf", name="b_f")
    nc.vector.tensor_mul(out=b_f, in0=maxe_pp, in1=zinv)
    b_bf = stat.tile([P, NIB], BF16, tag="bbf", name="b_bf")
    nc.vector.tensor_copy(out=b_bf, in_=b_f)

    # ---- h partial + AllReduce ------------------------------------------
    h_ps = zpool.tile([P, NDC], F32, tag="z", name="h_ps")
    for ib in range(NIB):
        for dc in range(NDC):
            nc.tensor.matmul(h_ps[:, dc:dc + 1],
                             lhsT=cb[:, ib, dc * P:(dc + 1) * P],
                             rhs=b_bf[:, ib:ib + 1],
                             start=(ib == 0 and dc == 0),
                             stop=(ib == NIB - 1 and dc == NDC - 1),
                             skip_group_check=True)
    h_sb = stat.tile([P, NDC], F32, tag="hsb", name="h_sb")
    nc.scalar.activation(out=h_sb, in_=h_ps, func=AF.Copy)
    hp_dram = dram.tile([D], F32)
    hs_dram = dram.tile([D], F32)
    hp_ap = hp_dram[:]
    nc.sync.dma_start(out=hp_ap.rearrange("(dc p) -> p dc", p=P), in_=h_sb)
    if collective:
        nc.gpsimd.collective_compute(
            "AllReduce", mybir.AluOpType.add,
            replica_groups=[list(range(NCORES))],
            ins=[hp_dram.opt()], outs=[hs_dram.opt()],
        )
    else:
        nc.sync.dma_start(out=hs_dram[:], in_=hp_dram[:])
    hs_ap = hs_dram[:]
    h_bc = consts.tile([P, D], F32)
    nc.sync.dma_start(
        out=h_bc,
        in_=bass.AP(tensor=hs_ap.tensor, offset=hs_ap.offset,
                    ap=[[0, P], [1, D]]),
    )

    # ---- U_A^T -> U_A, G1..2 (independent of h; fills the AR window) -----
    uat = consts.tile([P, NDC, TL], BF16)   # [p(dq), dqc, i]
    for dqc in range(NDC):
        nc.vector.tensor_copy(out=uat[:, dqc, :], in_=ua_ps[dqc])

    for ib in range(NIB):
        ps = spool.tile([P, D], BF16, tag="st", name=f"uat{ib}")
        for dqc in range(NDC):
            nc.tensor.transpose(ps[:, dqc * P:(dqc + 1) * P],
                                uat[:, dqc, ib * P:(ib + 1) * P], ident)
        g12 = gout.tile([P, 2, D], BF16, tag="g12", name=f"g12_{ib}")
        # G1 = U_A = U_A^T.T * zinv
        nc.scalar.activation(out=g12[:, 0, :], in_=ps, func=AF.Copy,
                             scale=zinv[:, ib:ib + 1])
        # G2 = c * U_A
        nc.vector.tensor_mul(out=g12[:, 1, :], in0=g12[:, 0, :],
                             in1=cb[:, ib, :])
        nc.sync.dma_start(out=g123_dram[ib * P:(ib + 1) * P, 0:2 * D],
                          in_=g12)

    # G0 = context, exact f32, DRAM->DRAM (independent; late so the startup
    # DMAs feed the matmul pipeline first)
    nc.sync.dma_start(out=g0_dram, in_=c_dram)

    # ---- G3 = c * h (the only h-dependent work) --------------------------
    hbc16 = stat.tile([P, D], BF16, tag="hbc16", name="hbc16")
    nc.vector.tensor_copy(out=hbc16, in_=h_bc)
    for ib in range(NIB):
        g3 = gout.tile([P, D], BF16, tag="g3", name=f"g3_{ib}")
        nc.vector.tensor_mul(out=g3, in0=cb[:, ib, :], in1=hbc16)
        nc.sync.dma_start(out=g123_dram[ib * P:(ib + 1) * P, 2 * D:3 * D],
                          in_=g3)

    ctx.close()


_NC_CACHE = {}


def _get_nc():
    if "nc" not in _NC_CACHE:
        _NC_CACHE["nc"] = build_kernel()
    return _NC_CACHE["nc"]


def _prep_inputs(x, kernel):
    """Host-side layout prep shared by kernel() and test harnesses."""
    context = np.ascontiguousarray(x[0, 0]).astype(np.float32)   # (T, D)
    question = np.ascontiguousarray(x[1, 0]).astype(np.float32)  # (T, D)
    w = np.asarray(kernel, dtype=np.float32)
    w2 = w[D:2 * D]
    w3 = w[2 * D:3 * D]
    w2p = np.ascontiguousarray(w2.reshape(NDC, P).T)
    w3p = np.ascontiguousarray(w3.reshape(NDC, P).T)

    q8 = question.astype(ml_dtypes.float8_e4m3)
    qlo8 = (question - q8.astype(np.float32)).astype(ml_dtypes.float8_e4m3)
    # q8n[p, jt, dq] = q8[jt*128 + p, dq]
    q8n = np.ascontiguousarray(q8.reshape(NJT, P, D).transpose(1, 0, 2))
    # qth[p, dc, j] = q8[j, dc*128 + p]
    qth = np.ascontiguousarray(q8.T.reshape(NDC, P, T).transpose(1, 0, 2))
    qtl = np.ascontiguousarray(qlo8.T.reshape(NDC, P, T).transpose(1, 0, 2))

    shared = {"q8n": q8n, "qth": qth, "qtl": qtl, "w2p": w2p, "w3p": w3p}
    in_maps = []
    for core in range(NCORES):
        m = dict(shared)
        m["c"] = np.ascontiguousarray(context[core * TL:(core + 1) * TL])
        in_maps.append(m)
    return in_maps


def _assemble(results):
    out = []
    for core in range(NCORES):
        g0 = np.asarray(results[core]["g0"], dtype=np.float32)
        g123 = np.asarray(results[core]["g123"]).astype(np.float32)
        out.append(np.concatenate([g0, g123], axis=1))
    return np.concatenate(out, axis=0)


def kernel(x: np.ndarray, kernel: np.ndarray) -> np.ndarray:
    nc = _get_nc()
    in_maps = _prep_inputs(x, kernel)
    res = run_bass_kernel_spmd(nc, in_maps, core_ids=list(range(NCORES)))
    return _assemble(res.results).astype(np.float32)
